# revision 16
# baseline (speedup 1.0000x reference)
"""Trainium2 Bass kernel for nn_LocalizationLoss (planar bf16 layout).

Loss (see reference):
  p = out[:,:,0]; t = tgt[:,:,0] in {0,1}
  bce  = -mean(t*ln(p) + (1-t)*ln(1-p)) = -mean ln|p + t - 1|
  trick= out * t[...,None]
  CE over slot axis (dim 1) of trick[:,:,4:7] with targets tgt[:,:,4]
  Lx   = mean((t*ox - tx)^2), Ly likewise
  Lwh  = mean((t*sqrt(ow) - sqrt(tw))^2) = mean(t*ow + tw - 2*t*sqrt(ow*tw))
  loss = 5*(Lx+Ly+2*Lwh) + bce + 0.5*(1-bce) + 3*ce

Strategy:
  - Host pre-shards along batch (8 cores), casts to bf16 and PLANARIZES:
    every channel becomes a dense per-partition plane, blocked by chunk so
    each chunk is one contiguous run per partition (single HWDGE DMA per
    chunk at HBM line rate; bf16 halves HBM traffic).
  - All device ops are unit-stride dense (DVE 2x bf16 tensor_tensor mode).
  - Plane order puts xy+logits contiguous so ONE tensor_tensor applies the
    presence mask to all 15 planes; logits stored j-major so t broadcasts
    uniformly. tw is shipped pre-masked (t*tw); sum(tw) is a pure-target
    scalar folded in on the host.
  - ACT engine does all transcendentals with fused accumulate-reductions;
    scalar_tensor_tensor provides fused compare/mult+accum for CE select.

Device sums per chunk (8 cols):
  BCE  = sum ln((p+t-1)^2 + 1e-6)            [host: * 0.5]
  SQXY = sum (t*ox-tx)^2 + (t*oy-ty)^2
  MW   = sum t*ow
  TS2  = sum 2*sqrt(t*ow*tw)  [= 2t*sqrt(ow*tw); exp(0.5*ln(mt)+ln2)]
  LSE  = sum_j ln sum_i exp(t_i*l_ij)
  SELi = sum_j (tgt_j==i) * t_i*l_ij
Host: s_wh = MW + sum(tw) - TS2
      loss = 0.5 + (5*SQXY + 10*s_wh - 0.25*BCE + 3*(LSE-sum SELi))/(3B)
"""

import numpy as np

import concourse.bass as bass
import concourse.bacc as bacc
import concourse.mybir as mybir
from concourse.tile import TileContext
from concourse.bass_utils import run_bass_kernel_spmd

# Force the ACT table pass to use only natural_log_exp_and_others (it holds
# every func this kernel needs: ln/exp/square/copy/identity). The default
# greedy per-func set choice thrashes between sets, costing a ~1.3us
# ACT_TABLE_LOAD each time. Blank the other sets, keep dict order so
# act_func_set_id indices stay aligned with act_info.json.
import concourse.hw_specs as _hw_specs
if not hasattr(_hw_specs, "_orig_get_activation_tables"):
    _hw_specs._orig_get_activation_tables = _hw_specs.get_activation_tables

    def _only_ln_exp_tables(module_arch):
        tabs = _hw_specs._orig_get_activation_tables(module_arch)
        return {
            name: (funcs if name == "natural_log_exp_and_others" else set())
            for name, funcs in tabs.items()
        }

    _hw_specs.get_activation_tables = _only_ln_exp_tables
    import concourse.bacc as _bacc_mod
    if hasattr(_bacc_mod, "get_activation_tables"):
        _bacc_mod.get_activation_tables = _only_ln_exp_tables

F32 = mybir.dt.float32
BF16 = mybir.dt.bfloat16
NP_BF16 = mybir.dt.np(BF16)
ALU = mybir.AluOpType
ACT = mybir.ActivationFunctionType
LN2 = 0.6931471805599453

P = 128          # SBUF partitions
N_CORES = 8
NPL = 36         # planes per b-group

# plane offsets (units of G) within a chunk tile
PL_P = 0         # p_i                      (3)
PL_XY = 3        # x_0..2, y_0..2           (6)
PL_L = 9         # l'_ji j-major            (9)
PL_W = 18        # w_i                      (3)
PL_T = 21        # t_i                      (3)
PL_TXY = 24      # tx_i, ty_i               (6)
PL_TWM = 30      # t_i*tw_i (pre-masked)    (3)
PL_TGT = 33      # tgt_j                    (3)

(COL_BCE, COL_SQXY, COL_MW, COL_TS2, COL_LSE,
 COL_SEL0, COL_SEL1, COL_SEL2) = range(8)
NCOL = 8

CHUNKS_FULL = (96, 224, 352, 352)   # sums to 1024 = nb/128


def build_kernel(g_total: int, chunks) -> bass.Bass:
    chunks = list(chunks)
    assert sum(chunks) == g_total, (sum(chunks), g_total)
    n_chunks = len(chunks)
    ncols = NCOL * n_chunks

    nc = bacc.Bacc()

    # Const [128,1] APs for activation bias values. The memsets are emitted
    # INSIDE the TileContext (first instructions) so Tile tracks the
    # memset->bias-read dependencies; no all_engine_barrier needed, which
    # lets the first input DMA issue immediately.
    const_tiles = {}
    for val in (-1.0, 1e-6, 1e-12, LN2):
        ctile = nc.alloc_sbuf_tensor(f"const-f32-{val}", [128, 1], F32)
        nc.const_aps.aps[(F32, val)] = ctile.ap()
        const_tiles[val] = ctile

    data_hbm = nc.declare_dram_parameter(
        "data", [P * NPL * g_total], BF16, isOutput=False)
    res_hbm = nc.declare_dram_parameter("res", [P, ncols], F32, isOutput=True)

    data_v = data_hbm[:].rearrange("(p n) -> p n", p=P)

    with TileContext(nc) as tc:
        with (
            tc.tile_pool(name="io", bufs=2) as io_pool,
            tc.tile_pool(name="mid", bufs=2) as mid_pool,
            tc.tile_pool(name="junk", bufs=1) as junk_pool,
            tc.tile_pool(name="accp", bufs=1) as acc_pool,
        ):
            for val, ctile in const_tiles.items():
                nc.gpsimd.memset(ctile.ap(), val)

            cols = acc_pool.tile([P, ncols], F32)
            off0 = 0
            # Ln(S) of chunk c is deferred into chunk c+1's ACT stream (first
            # ACT op there) so ACT never stalls on the exp->sum->ln chain.
            pending = []

            def emit_lnS():
                cb_p, S_p, jS_p = pending.pop(0)
                nc.scalar.activation(
                    jS_p[:, :], S_p[:, :], ACT.Ln,
                    accum_out=cols[:, cb_p + COL_LSE:cb_p + COL_LSE + 1],
                )
                # this chunk's col block is now complete; ship it out on the
                # (idle) SWDGE queue so the sync queue stays clear for loads
                nc.gpsimd.dma_start(
                    out=res_hbm[:, cb_p:cb_p + NCOL],
                    in_=cols[:, cb_p:cb_p + NCOL],
                )

            for c, G in enumerate(chunks):
                cb = c * NCOL

                tile = io_pool.tile([P, NPL * G], BF16, tag="tile")
                nc.sync.dma_start(
                    out=tile[:, :],
                    in_=data_v[:, off0:off0 + NPL * G],
                )
                off0 += NPL * G

                def pl(a, b):
                    return tile[:, a * G:b * G]

                P3 = pl(PL_P, PL_P + 3)
                XYL15 = pl(PL_XY, PL_XY + 15)
                W3 = pl(PL_W, PL_W + 3)
                T3 = pl(PL_T, PL_T + 3)
                TXY6 = pl(PL_TXY, PL_TXY + 6)
                TWM3 = pl(PL_TWM, PL_TWM + 3)
                TGT3 = pl(PL_TGT, PL_TGT + 3)

                # t_i broadcast over the 5 (c-)groups of xy+logits planes
                t_b15 = (
                    T3.rearrange("p (c i g) -> p c i g", c=1, i=3)
                    .broadcast_to([P, 5, 3, G])
                )

                # ---- scratch ----
                qs = mid_pool.tile([P, 3 * G], BF16, tag="qs")
                qsq = mid_pool.tile([P, 3 * G], BF16, tag="qsq")
                M15 = mid_pool.tile([P, 15 * G], BF16, tag="M15")
                E = mid_pool.tile([P, 9 * G], BF16, tag="E")
                S = mid_pool.tile([P, 3 * G], BF16, tag="S")
                exy = mid_pool.tile([P, 6 * G], BF16, tag="exy")
                mt = mid_pool.tile([P, 3 * G], BF16, tag="mt")
                lm = mid_pool.tile([P, 3 * G], F32, tag="lm")
                jb = junk_pool.tile([P, 3 * G], BF16, tag="jb")
                jsq = junk_pool.tile([P, 6 * G], BF16, tag="jsq")
                jwh = junk_pool.tile([P, 3 * G], BF16, tag="jwh")
                jS = junk_pool.tile([P, 3 * G], BF16, tag="jS")
                jmw = junk_pool.tile([P, 3 * G], BF16, tag="jmw")
                jsel = junk_pool.tile([P, 3 * G], BF16, tag="jsel")

                M15_v = M15[:, :].rearrange("p (c i g) -> p c i g", c=5, i=3)
                XYL15_v = XYL15.rearrange("p (c i g) -> p c i g", c=5, i=3)
                Mxy = M15[:, 0:6 * G]
                Mlog = M15[:, 6 * G:15 * G]          # masked logits, j-major
                Mlog_v = Mlog.rearrange("p (j i g) -> p j i g", j=3, i=3)
                E_v = E[:, :].rearrange("p (j i g) -> p j i g", j=3, i=3)
                S_v = S[:, :].rearrange("p (j g) -> p j g", j=3)

                # ---- DVE head: u = p + t ; masked xy+logits ----
                nc.vector.tensor_tensor(qs[:, :], P3, T3, ALU.add)
                nc.vector.tensor_tensor(M15_v, XYL15_v, t_b15, ALU.mult)

                # ---- ACT stream: prev chunk's ln(S), then this chunk ----
                if pending:
                    emit_lnS()
                nc.scalar.activation(qsq[:, :], qs[:, :], ACT.Square,
                                     bias=-1.0, scale=1.0)
                nc.scalar.activation(
                    jb[:, :], qsq[:, :], ACT.Ln, bias=1e-6, scale=1.0,
                    accum_out=cols[:, cb + COL_BCE:cb + COL_BCE + 1],
                )
                nc.scalar.activation(E[:, :], Mlog, ACT.Exp)

                # ---- wh: col_MW += sum t*ow (fused accum via STT) ----
                nc.vector.scalar_tensor_tensor(
                    jmw[:, :], W3, 1.0, T3, ALU.mult, ALU.mult,
                    accum_out=cols[:, cb + COL_MW:cb + COL_MW + 1],
                )
                # mt = ow * (t*tw)
                nc.vector.tensor_tensor(mt[:, :], W3, TWM3, ALU.mult)
                nc.scalar.activation(lm[:, :], mt[:, :], ACT.Ln, bias=1e-12)
                nc.scalar.activation(
                    jwh[:, :], lm[:, :], ACT.Exp, bias=LN2, scale=0.5,
                    accum_out=cols[:, cb + COL_TS2:cb + COL_TS2 + 1],
                )

                # ---- xy MSE ----
                nc.vector.tensor_tensor(exy[:, :], Mxy, TXY6, ALU.subtract)
                nc.scalar.activation(
                    jsq[:, :], exy[:, :], ACT.Square,
                    accum_out=cols[:, cb + COL_SQXY:cb + COL_SQXY + 1],
                )

                # ---- CE select ----
                for i in range(3):
                    nc.vector.scalar_tensor_tensor(
                        jsel[:, :], TGT3, float(i), Mlog_v[:, :, i],
                        ALU.is_equal, ALU.mult,
                        accum_out=cols[:, cb + COL_SEL0 + i:cb + COL_SEL0 + i + 1],
                    )

                # ---- CE tail: S = sum_i E (inline; exp has finished by now)
                nc.vector.tensor_tensor(
                    S_v, E_v[:, :, 0], E_v[:, :, 1], ALU.add)
                nc.vector.tensor_tensor(S_v, S_v, E_v[:, :, 2], ALU.add)
                pending.append((cb, S, jS))

            while pending:
                emit_lnS()

    nc.compile()
    return nc


def _chunks_for(g_total: int):
    if g_total == 1024:
        return CHUNKS_FULL
    for n in (4, 2, 1):
        if g_total % n == 0:
            return (g_total // n,) * n
    return (g_total,)


def planarize(o_shard: np.ndarray, t_shard: np.ndarray, chunks) -> np.ndarray:
    """(nb,3,7)+(nb,3,5) f32 -> flat [P*NPL*g_total] bf16, chunk-blocked."""
    nbb = o_shard.shape[0]
    gt = nbb // P
    ob = o_shard.reshape(P, gt, 3, 7)
    tb = t_shard.reshape(P, gt, 3, 5)
    planes = np.empty((P, NPL, gt), dtype=NP_BF16)
    op = ob.transpose(0, 3, 2, 1)                     # (P, 7c, 3i, gt)
    planes[:, 0:9] = op[:, 0:3].reshape(P, 9, gt)     # p, x, y
    planes[:, 9:18] = (
        ob[:, :, :, 4:7].transpose(0, 3, 2, 1).reshape(P, 9, gt))  # l j-major
    planes[:, 18:21] = op[:, 3]                       # w
    tp = tb.transpose(0, 3, 2, 1)                     # (P, 5c, 3i, gt)
    planes[:, 21:30] = tp[:, 0:3].reshape(P, 9, gt)   # t, tx, ty
    planes[:, 30:33] = tp[:, 0] * tp[:, 3]            # t*tw
    planes[:, 33:36] = tb[:, :, :, 4].transpose(0, 2, 1)           # tgt_j
    parts = []
    g0 = 0
    for G in chunks:
        parts.append(np.ascontiguousarray(planes[:, :, g0:g0 + G]).reshape(P, -1))
        g0 += G
    return np.concatenate(parts, axis=1).ravel()


def make_in_maps(output: np.ndarray, target: np.ndarray, chunks):
    b = output.shape[0]
    nb = b // N_CORES
    in_maps = []
    for k in range(N_CORES):
        data = planarize(output[k * nb:(k + 1) * nb],
                         target[k * nb:(k + 1) * nb], chunks)
        in_maps.append({"data": data})
    return in_maps


def host_tw_sum(target: np.ndarray) -> float:
    """Pure-target partial sum folded in on the host: sum of tw (bf16-cast,
    matching what the device would have seen)."""
    return float(
        target[:, :, 3].astype(NP_BF16).astype(np.float64).sum())


def combine_results(res_list, n_chunks: int, b_total: int,
                    s_tw: float) -> np.float32:
    acc = np.zeros(NCOL, dtype=np.float64)
    for res in res_list:
        r = np.asarray(res).astype(np.float64).reshape(P, n_chunks, NCOL)
        acc += r.sum(axis=(0, 1))
    s_wh = acc[COL_MW] + s_tw - acc[COL_TS2]
    s_sel = acc[COL_SEL0] + acc[COL_SEL1] + acc[COL_SEL2]
    denom = 3.0 * b_total
    loss = 0.5 + (
        5.0 * acc[COL_SQXY] + 10.0 * s_wh - 0.25 * acc[COL_BCE]
        + 3.0 * (acc[COL_LSE] - s_sel)
    ) / denom
    return np.float32(loss)


_CACHED = {}


def _get_nc(nb: int):
    g_total = nb // P
    chunks = _chunks_for(g_total)
    key = (g_total, chunks)
    if key not in _CACHED:
        _CACHED[key] = (build_kernel(g_total, chunks), chunks)
    return _CACHED[key]


def run_on_cores(output: np.ndarray, target: np.ndarray, trace: bool = False):
    b = output.shape[0]
    nb = b // N_CORES
    nc, chunks = _get_nc(nb)
    in_maps = make_in_maps(output, target, chunks)
    results = run_bass_kernel_spmd(
        nc, in_maps, core_ids=list(range(N_CORES)), trace=trace
    )
    res_list = [r["res"] for r in results.results]
    return res_list, len(chunks), results


def kernel(output: np.ndarray, target: np.ndarray) -> np.ndarray:
    output = np.asarray(output, dtype=np.float32)
    target = np.asarray(target, dtype=np.float32)
    b = output.shape[0]
    res_list, n_chunks, _ = run_on_cores(output, target)
    return combine_results(res_list, n_chunks=n_chunks, b_total=b,
                           s_tw=host_tw_sum(target))


# revision 18
# speedup vs baseline: 1.0399x; 1.0399x over previous
"""Trainium2 Bass kernel for nn_LocalizationLoss (planar bf16 layout).

Loss (see reference):
  p = out[:,:,0]; t = tgt[:,:,0] in {0,1}
  bce  = -mean(t*ln(p) + (1-t)*ln(1-p)) = -mean ln|p + t - 1|
  trick= out * t[...,None]
  CE over slot axis (dim 1) of trick[:,:,4:7] with targets tgt[:,:,4]
  Lx   = mean((t*ox - tx)^2), Ly likewise
  Lwh  = mean((t*sqrt(ow) - sqrt(tw))^2) = mean(t*ow + tw - 2*t*sqrt(ow*tw))
  loss = 5*(Lx+Ly+2*Lwh) + bce + 0.5*(1-bce) + 3*ce

Strategy:
  - Host pre-shards along batch (8 cores), casts to bf16 and PLANARIZES:
    every channel becomes a dense per-partition plane, blocked by chunk so
    each chunk is one contiguous run per partition (single HWDGE DMA per
    chunk at HBM line rate; bf16 halves HBM traffic).
  - All device ops are unit-stride dense (DVE 2x bf16 tensor_tensor mode).
  - Plane order puts xy+logits contiguous so ONE tensor_tensor applies the
    presence mask to all 15 planes; logits stored j-major so t broadcasts
    uniformly. tw is shipped pre-masked (t*tw); sum(tw) is a pure-target
    scalar folded in on the host.
  - ACT engine does all transcendentals with fused accumulate-reductions;
    scalar_tensor_tensor provides fused compare/mult+accum for CE select.

Device sums per chunk (8 cols):
  BCE  = sum ln((p+t-1)^2 + 1e-6)            [host: * 0.5]
  SQXY = sum (t*ox-tx)^2 + (t*oy-ty)^2
  MW   = sum t*ow
  TS2  = sum 2*sqrt(t*ow*tw)  [= 2t*sqrt(ow*tw); exp(0.5*ln(mt)+ln2)]
  LSE  = sum_j ln sum_i exp(t_i*l_ij)
  SELi = sum_j (tgt_j==i) * t_i*l_ij
Host: s_wh = MW + sum(tw) - TS2
      loss = 0.5 + (5*SQXY + 10*s_wh - 0.25*BCE + 3*(LSE-sum SELi))/(3B)
"""

import numpy as np

import concourse.bass as bass
import concourse.bacc as bacc
import concourse.mybir as mybir
from concourse.tile import TileContext
from concourse.bass_utils import run_bass_kernel_spmd

# Force the ACT table pass to use only natural_log_exp_and_others (it holds
# every func this kernel needs: ln/exp/square/copy/identity). The default
# greedy per-func set choice thrashes between sets, costing a ~1.3us
# ACT_TABLE_LOAD each time. Blank the other sets, keep dict order so
# act_func_set_id indices stay aligned with act_info.json.
import concourse.hw_specs as _hw_specs
if not hasattr(_hw_specs, "_orig_get_activation_tables"):
    _hw_specs._orig_get_activation_tables = _hw_specs.get_activation_tables

    def _only_ln_exp_tables(module_arch):
        tabs = _hw_specs._orig_get_activation_tables(module_arch)
        return {
            name: (funcs if name == "natural_log_exp_and_others" else set())
            for name, funcs in tabs.items()
        }

    _hw_specs.get_activation_tables = _only_ln_exp_tables
    import concourse.bacc as _bacc_mod
    if hasattr(_bacc_mod, "get_activation_tables"):
        _bacc_mod.get_activation_tables = _only_ln_exp_tables

F32 = mybir.dt.float32
BF16 = mybir.dt.bfloat16
NP_BF16 = mybir.dt.np(BF16)
ALU = mybir.AluOpType
ACT = mybir.ActivationFunctionType
LN2 = 0.6931471805599453

P = 128          # SBUF partitions
N_CORES = 8
NPL = 36         # planes per b-group

# plane offsets (units of G) within a chunk tile
PL_P = 0         # p_i                      (3)
PL_XY = 3        # x_0..2, y_0..2           (6)
PL_L = 9         # l'_ji j-major            (9)
PL_W = 18        # w_i                      (3)
PL_T = 21        # t_i                      (3)
PL_TXY = 24      # tx_i, ty_i               (6)
PL_TWM = 30      # t_i*tw_i (pre-masked)    (3)
PL_TGT = 33      # tgt_j                    (3)

(COL_BCE, COL_SQXY, COL_MW, COL_TS2, COL_LSE,
 COL_SEL0, COL_SEL1, COL_SEL2) = range(8)
NCOL = 8

CHUNKS_FULL = (96, 224, 352, 352)   # sums to 1024 = nb/128


def build_kernel(g_total: int, chunks) -> bass.Bass:
    chunks = list(chunks)
    assert sum(chunks) == g_total, (sum(chunks), g_total)
    n_chunks = len(chunks)
    ncols = NCOL * n_chunks

    nc = bacc.Bacc()

    # Const [128,1] APs for activation bias values. The memsets are emitted
    # INSIDE the TileContext (first instructions) so Tile tracks the
    # memset->bias-read dependencies; no all_engine_barrier needed, which
    # lets the first input DMA issue immediately.
    const_tiles = {}
    for val in (-1.0, 1e-6, 1e-12, LN2):
        ctile = nc.alloc_sbuf_tensor(f"const-f32-{val}", [128, 1], F32)
        nc.const_aps.aps[(F32, val)] = ctile.ap()
        const_tiles[val] = ctile

    data_hbm = nc.declare_dram_parameter(
        "data", [P * NPL * g_total], BF16, isOutput=False)
    res_hbm = nc.declare_dram_parameter("res", [P, ncols], F32, isOutput=True)

    data_v = data_hbm[:].rearrange("(p n) -> p n", p=P)

    with TileContext(nc) as tc:
        with (
            tc.tile_pool(name="io", bufs=2) as io_pool,
            tc.tile_pool(name="mid", bufs=2) as mid_pool,
            tc.tile_pool(name="junk", bufs=1) as junk_pool,
            tc.tile_pool(name="accp", bufs=1) as acc_pool,
        ):
            for val, ctile in const_tiles.items():
                nc.gpsimd.memset(ctile.ap(), val)

            cols = acc_pool.tile([P, ncols], F32)
            off0 = 0
            # Ln(S) of chunk c is deferred into chunk c+1's ACT stream (first
            # ACT op there) so ACT never stalls on the exp->sum->ln chain.
            pending = []

            def emit_lnS():
                cb_p, S_p, jS_p = pending.pop(0)
                nc.scalar.activation(
                    jS_p[:, :], S_p[:, :], ACT.Ln,
                    accum_out=cols[:, cb_p + COL_LSE:cb_p + COL_LSE + 1],
                )


            for c, G in enumerate(chunks):
                cb = c * NCOL

                tile = io_pool.tile([P, NPL * G], BF16, tag="tile")
                nc.sync.dma_start(
                    out=tile[:, :],
                    in_=data_v[:, off0:off0 + NPL * G],
                )
                off0 += NPL * G

                def pl(a, b):
                    return tile[:, a * G:b * G]

                P3 = pl(PL_P, PL_P + 3)
                XYL15 = pl(PL_XY, PL_XY + 15)
                W3 = pl(PL_W, PL_W + 3)
                T3 = pl(PL_T, PL_T + 3)
                TXY6 = pl(PL_TXY, PL_TXY + 6)
                TWM3 = pl(PL_TWM, PL_TWM + 3)
                TGT3 = pl(PL_TGT, PL_TGT + 3)

                # t_i broadcast over the 5 (c-)groups of xy+logits planes
                t_b15 = (
                    T3.rearrange("p (c i g) -> p c i g", c=1, i=3)
                    .broadcast_to([P, 5, 3, G])
                )

                # ---- scratch ----
                qs = mid_pool.tile([P, 3 * G], BF16, tag="qs")
                qsq = mid_pool.tile([P, 3 * G], BF16, tag="qsq")
                M15 = mid_pool.tile([P, 15 * G], BF16, tag="M15")
                E = mid_pool.tile([P, 9 * G], BF16, tag="E")
                S = mid_pool.tile([P, 3 * G], BF16, tag="S")
                exy = mid_pool.tile([P, 6 * G], BF16, tag="exy")
                mt = mid_pool.tile([P, 3 * G], BF16, tag="mt")
                lm = mid_pool.tile([P, 3 * G], F32, tag="lm")
                jb = junk_pool.tile([P, 3 * G], BF16, tag="jb")
                jsq = junk_pool.tile([P, 6 * G], BF16, tag="jsq")
                jwh = junk_pool.tile([P, 3 * G], BF16, tag="jwh")
                jS = junk_pool.tile([P, 3 * G], BF16, tag="jS")
                jmw = junk_pool.tile([P, 3 * G], BF16, tag="jmw")
                jsel = junk_pool.tile([P, 3 * G], BF16, tag="jsel")

                M15_v = M15[:, :].rearrange("p (c i g) -> p c i g", c=5, i=3)
                XYL15_v = XYL15.rearrange("p (c i g) -> p c i g", c=5, i=3)
                Mxy = M15[:, 0:6 * G]
                Mlog = M15[:, 6 * G:15 * G]          # masked logits, j-major
                Mlog_v = Mlog.rearrange("p (j i g) -> p j i g", j=3, i=3)
                E_v = E[:, :].rearrange("p (j i g) -> p j i g", j=3, i=3)
                S_v = S[:, :].rearrange("p (j g) -> p j g", j=3)

                # ---- DVE head: u = p + t ; masked xy+logits ----
                nc.vector.tensor_tensor(qs[:, :], P3, T3, ALU.add)
                nc.vector.tensor_tensor(M15_v, XYL15_v, t_b15, ALU.mult)

                # ---- ACT stream: prev chunk's ln(S), then this chunk ----
                if pending:
                    emit_lnS()
                nc.scalar.activation(qsq[:, :], qs[:, :], ACT.Square,
                                     bias=-1.0, scale=1.0)
                nc.scalar.activation(
                    jb[:, :], qsq[:, :], ACT.Ln, bias=1e-6, scale=1.0,
                    accum_out=cols[:, cb + COL_BCE:cb + COL_BCE + 1],
                )
                nc.scalar.activation(E[:, :], Mlog, ACT.Exp)

                # ---- wh: col_MW += sum t*ow (fused accum via STT) ----
                nc.vector.scalar_tensor_tensor(
                    jmw[:, :], W3, 1.0, T3, ALU.mult, ALU.mult,
                    accum_out=cols[:, cb + COL_MW:cb + COL_MW + 1],
                )
                # mt = ow * (t*tw)
                nc.vector.tensor_tensor(mt[:, :], W3, TWM3, ALU.mult)
                nc.scalar.activation(lm[:, :], mt[:, :], ACT.Ln, bias=1e-12)
                nc.scalar.activation(
                    jwh[:, :], lm[:, :], ACT.Exp, bias=LN2, scale=0.5,
                    accum_out=cols[:, cb + COL_TS2:cb + COL_TS2 + 1],
                )

                # ---- xy MSE ----
                nc.vector.tensor_tensor(exy[:, :], Mxy, TXY6, ALU.subtract)
                nc.scalar.activation(
                    jsq[:, :], exy[:, :], ACT.Square,
                    accum_out=cols[:, cb + COL_SQXY:cb + COL_SQXY + 1],
                )

                # ---- CE select ----
                for i in range(3):
                    nc.vector.scalar_tensor_tensor(
                        jsel[:, :], TGT3, float(i), Mlog_v[:, :, i],
                        ALU.is_equal, ALU.mult,
                        accum_out=cols[:, cb + COL_SEL0 + i:cb + COL_SEL0 + i + 1],
                    )

                # ---- CE tail: S = sum_i E (inline; exp has finished by now)
                nc.vector.tensor_tensor(
                    S_v, E_v[:, :, 0], E_v[:, :, 1], ALU.add)
                nc.vector.tensor_tensor(S_v, S_v, E_v[:, :, 2], ALU.add)
                pending.append((cb, S, jS))

            while pending:
                emit_lnS()

            nc.sync.dma_start(out=res_hbm[:, :], in_=cols[:, :])

    nc.compile()
    return nc


def _chunks_for(g_total: int):
    if g_total == 1024:
        return CHUNKS_FULL
    for n in (4, 2, 1):
        if g_total % n == 0:
            return (g_total // n,) * n
    return (g_total,)


def planarize(o_shard: np.ndarray, t_shard: np.ndarray, chunks) -> np.ndarray:
    """(nb,3,7)+(nb,3,5) f32 -> flat [P*NPL*g_total] bf16, chunk-blocked."""
    nbb = o_shard.shape[0]
    gt = nbb // P
    ob = o_shard.reshape(P, gt, 3, 7)
    tb = t_shard.reshape(P, gt, 3, 5)
    planes = np.empty((P, NPL, gt), dtype=NP_BF16)
    op = ob.transpose(0, 3, 2, 1)                     # (P, 7c, 3i, gt)
    planes[:, 0:9] = op[:, 0:3].reshape(P, 9, gt)     # p, x, y
    planes[:, 9:18] = (
        ob[:, :, :, 4:7].transpose(0, 3, 2, 1).reshape(P, 9, gt))  # l j-major
    planes[:, 18:21] = op[:, 3]                       # w
    tp = tb.transpose(0, 3, 2, 1)                     # (P, 5c, 3i, gt)
    planes[:, 21:30] = tp[:, 0:3].reshape(P, 9, gt)   # t, tx, ty
    planes[:, 30:33] = tp[:, 0] * tp[:, 3]            # t*tw
    planes[:, 33:36] = tb[:, :, :, 4].transpose(0, 2, 1)           # tgt_j
    parts = []
    g0 = 0
    for G in chunks:
        parts.append(np.ascontiguousarray(planes[:, :, g0:g0 + G]).reshape(P, -1))
        g0 += G
    return np.concatenate(parts, axis=1).ravel()


def make_in_maps(output: np.ndarray, target: np.ndarray, chunks):
    b = output.shape[0]
    nb = b // N_CORES
    in_maps = []
    for k in range(N_CORES):
        data = planarize(output[k * nb:(k + 1) * nb],
                         target[k * nb:(k + 1) * nb], chunks)
        in_maps.append({"data": data})
    return in_maps


def host_tw_sum(target: np.ndarray) -> float:
    """Pure-target partial sum folded in on the host: sum of tw (bf16-cast,
    matching what the device would have seen)."""
    return float(
        target[:, :, 3].astype(NP_BF16).astype(np.float64).sum())


def combine_results(res_list, n_chunks: int, b_total: int,
                    s_tw: float) -> np.float32:
    acc = np.zeros(NCOL, dtype=np.float64)
    for res in res_list:
        r = np.asarray(res).astype(np.float64).reshape(P, n_chunks, NCOL)
        acc += r.sum(axis=(0, 1))
    s_wh = acc[COL_MW] + s_tw - acc[COL_TS2]
    s_sel = acc[COL_SEL0] + acc[COL_SEL1] + acc[COL_SEL2]
    denom = 3.0 * b_total
    loss = 0.5 + (
        5.0 * acc[COL_SQXY] + 10.0 * s_wh - 0.25 * acc[COL_BCE]
        + 3.0 * (acc[COL_LSE] - s_sel)
    ) / denom
    return np.float32(loss)


_CACHED = {}


def _get_nc(nb: int):
    g_total = nb // P
    chunks = _chunks_for(g_total)
    key = (g_total, chunks)
    if key not in _CACHED:
        _CACHED[key] = (build_kernel(g_total, chunks), chunks)
    return _CACHED[key]


def run_on_cores(output: np.ndarray, target: np.ndarray, trace: bool = False):
    b = output.shape[0]
    nb = b // N_CORES
    nc, chunks = _get_nc(nb)
    in_maps = make_in_maps(output, target, chunks)
    results = run_bass_kernel_spmd(
        nc, in_maps, core_ids=list(range(N_CORES)), trace=trace
    )
    res_list = [r["res"] for r in results.results]
    return res_list, len(chunks), results


def kernel(output: np.ndarray, target: np.ndarray) -> np.ndarray:
    output = np.asarray(output, dtype=np.float32)
    target = np.asarray(target, dtype=np.float32)
    b = output.shape[0]
    res_list, n_chunks, _ = run_on_cores(output, target)
    return combine_results(res_list, n_chunks=n_chunks, b_total=b,
                           s_tw=host_tw_sum(target))


# revision 22
# speedup vs baseline: 1.0673x; 1.0264x over previous
"""Trainium2 Bass kernel for nn_LocalizationLoss (planar bf16 layout).

Loss (see reference):
  p = out[:,:,0]; t = tgt[:,:,0] in {0,1}
  bce  = -mean(t*ln(p) + (1-t)*ln(1-p)) = -mean ln|p + t - 1|
  trick= out * t[...,None]
  CE over slot axis (dim 1) of trick[:,:,4:7] with targets tgt[:,:,4]
  Lx   = mean((t*ox - tx)^2), Ly likewise
  Lwh  = mean((t*sqrt(ow) - sqrt(tw))^2) = mean(t*ow + tw - 2*t*sqrt(ow*tw))
  loss = 5*(Lx+Ly+2*Lwh) + bce + 0.5*(1-bce) + 3*ce

Strategy:
  - Host pre-shards along batch (8 cores), casts to bf16 and PLANARIZES:
    every channel becomes a dense per-partition plane, blocked by chunk so
    each chunk is one contiguous run per partition (single HWDGE DMA per
    chunk at HBM line rate; bf16 halves HBM traffic).
  - All device ops are unit-stride dense (DVE 2x bf16 tensor_tensor mode).
  - Plane order puts xy+logits contiguous so ONE tensor_tensor applies the
    presence mask to all 15 planes; logits stored j-major so t broadcasts
    uniformly. tw is shipped pre-masked (t*tw); sum(tw) is a pure-target
    scalar folded in on the host.
  - ACT engine does all transcendentals with fused accumulate-reductions;
    scalar_tensor_tensor provides fused compare/mult+accum for CE select.

Device sums per chunk (8 cols):
  BCE  = sum ln((p+t-1)^2 + 1e-6)            [host: * 0.5]
  SQXY = sum (t*ox-tx)^2 + (t*oy-ty)^2
  MW   = sum t*ow
  TS2  = sum 2*sqrt(t*ow*tw)  [= 2t*sqrt(ow*tw); exp(0.5*ln(mt)+ln2)]
  LSE  = sum_j ln sum_i exp(t_i*l_ij)
  SELi = sum_j (tgt_j==i) * t_i*l_ij
Host: s_wh = MW + sum(tw) - TS2
      loss = 0.5 + (5*SQXY + 10*s_wh - 0.25*BCE + 3*(LSE-sum SELi))/(3B)
"""

import numpy as np

import concourse.bass as bass
import concourse.bacc as bacc
import concourse.mybir as mybir
from concourse.tile import TileContext
from concourse.bass_utils import run_bass_kernel_spmd

# Force the ACT table pass to use only natural_log_exp_and_others (it holds
# every func this kernel needs: ln/exp/square/copy/identity). The default
# greedy per-func set choice thrashes between sets, costing a ~1.3us
# ACT_TABLE_LOAD each time. Blank the other sets, keep dict order so
# act_func_set_id indices stay aligned with act_info.json.
import concourse.hw_specs as _hw_specs
if not hasattr(_hw_specs, "_orig_get_activation_tables"):
    _hw_specs._orig_get_activation_tables = _hw_specs.get_activation_tables

    def _only_ln_exp_tables(module_arch):
        tabs = _hw_specs._orig_get_activation_tables(module_arch)
        return {
            name: (funcs if name == "natural_log_exp_and_others" else set())
            for name, funcs in tabs.items()
        }

    _hw_specs.get_activation_tables = _only_ln_exp_tables
    import concourse.bacc as _bacc_mod
    if hasattr(_bacc_mod, "get_activation_tables"):
        _bacc_mod.get_activation_tables = _only_ln_exp_tables

F32 = mybir.dt.float32
BF16 = mybir.dt.bfloat16
NP_BF16 = mybir.dt.np(BF16)
ALU = mybir.AluOpType
ACT = mybir.ActivationFunctionType
LN2 = 0.6931471805599453

P = 128          # SBUF partitions
N_CORES = 8
NPL = 36         # planes per b-group

# plane offsets (units of G) within a chunk tile
PL_P = 0         # p_i                      (3)
PL_XY = 3        # x_0..2, y_0..2           (6)
PL_L = 9         # l'_ji j-major            (9)
PL_W = 18        # w_i                      (3)
PL_T = 21        # t_i                      (3)
PL_TXY = 24      # tx_i, ty_i               (6)
PL_TWM = 30      # t_i*tw_i (pre-masked)    (3)
PL_TGT = 33      # tgt_j                    (3)

(COL_BCE, COL_SQXY, COL_MW, COL_TS2, COL_LSE,
 COL_SEL0, COL_SEL1, COL_SEL2) = range(8)
NCOL = 8

CHUNKS_FULL = (64, 256, 352, 352)   # sums to 1024 = nb/128


def build_kernel(g_total: int, chunks) -> bass.Bass:
    chunks = list(chunks)
    assert sum(chunks) == g_total, (sum(chunks), g_total)
    n_chunks = len(chunks)
    ncols = NCOL * n_chunks

    nc = bacc.Bacc()

    # Const [128,1] APs for activation bias values. The memsets are emitted
    # INSIDE the TileContext (first instructions) so Tile tracks the
    # memset->bias-read dependencies; no all_engine_barrier needed, which
    # lets the first input DMA issue immediately.
    const_tiles = {}
    for val in (-1.0, 1e-6, 1e-12, LN2):
        ctile = nc.alloc_sbuf_tensor(f"const-f32-{val}", [128, 1], F32)
        nc.const_aps.aps[(F32, val)] = ctile.ap()
        const_tiles[val] = ctile

    data_hbm = nc.declare_dram_parameter(
        "data", [P * NPL * g_total], BF16, isOutput=False)
    res_hbm = nc.declare_dram_parameter("res", [P, ncols], F32, isOutput=True)

    data_v = data_hbm[:].rearrange("(p n) -> p n", p=P)

    with TileContext(nc) as tc:
        with (
            tc.tile_pool(name="io", bufs=2) as io_pool,
            tc.tile_pool(name="mid", bufs=2) as mid_pool,
            tc.tile_pool(name="junk", bufs=1) as junk_pool,
            tc.tile_pool(name="accp", bufs=1) as acc_pool,
        ):
            for val, ctile in const_tiles.items():
                nc.gpsimd.memset(ctile.ap(), val)

            cols = acc_pool.tile([P, ncols], F32)

            # Dummy tiny ACT op: triggers the ACT_TABLE_LOAD during the
            # preamble so the first real activation doesn't wait ~2.7us.
            warm = acc_pool.tile([P, 1], F32)
            nc.scalar.activation(warm[:, :], const_tiles[LN2].ap(), ACT.Exp)

            off0 = 0
            # Ln(S) of chunk c is deferred into chunk c+1's ACT stream (first
            # ACT op there) so ACT never stalls on the exp->sum->ln chain.
            pending = []

            def emit_lnS():
                cb_p, S_p, jS_p = pending.pop(0)
                nc.scalar.activation(
                    jS_p[:, :], S_p[:, :], ACT.Ln,
                    accum_out=cols[:, cb_p + COL_LSE:cb_p + COL_LSE + 1],
                )


            for c, G in enumerate(chunks):
                cb = c * NCOL

                tile = io_pool.tile([P, NPL * G], BF16, tag="tile")
                nc.sync.dma_start(
                    out=tile[:, :],
                    in_=data_v[:, off0:off0 + NPL * G],
                )
                off0 += NPL * G

                def pl(a, b):
                    return tile[:, a * G:b * G]

                P3 = pl(PL_P, PL_P + 3)
                XYL15 = pl(PL_XY, PL_XY + 15)
                W3 = pl(PL_W, PL_W + 3)
                T3 = pl(PL_T, PL_T + 3)
                TXY6 = pl(PL_TXY, PL_TXY + 6)
                TWM3 = pl(PL_TWM, PL_TWM + 3)
                TGT3 = pl(PL_TGT, PL_TGT + 3)

                # t_i broadcast over the 5 (c-)groups of xy+logits planes
                t_b15 = (
                    T3.rearrange("p (c i g) -> p c i g", c=1, i=3)
                    .broadcast_to([P, 5, 3, G])
                )

                # ---- scratch ----
                qs = mid_pool.tile([P, 3 * G], BF16, tag="qs")
                qsq = mid_pool.tile([P, 3 * G], BF16, tag="qsq")
                M15 = mid_pool.tile([P, 15 * G], BF16, tag="M15")
                E = mid_pool.tile([P, 9 * G], BF16, tag="E")
                S = mid_pool.tile([P, 3 * G], BF16, tag="S")
                exy = mid_pool.tile([P, 6 * G], BF16, tag="exy")
                mt = mid_pool.tile([P, 3 * G], BF16, tag="mt")
                lm = mid_pool.tile([P, 3 * G], F32, tag="lm")
                jb = junk_pool.tile([P, 3 * G], BF16, tag="jb")
                jsq = junk_pool.tile([P, 6 * G], BF16, tag="jsq")
                jwh = junk_pool.tile([P, 3 * G], BF16, tag="jwh")
                jS = junk_pool.tile([P, 3 * G], BF16, tag="jS")
                jmw = junk_pool.tile([P, 3 * G], BF16, tag="jmw")
                jsel = junk_pool.tile([P, 3 * G], BF16, tag="jsel")

                M15_v = M15[:, :].rearrange("p (c i g) -> p c i g", c=5, i=3)
                XYL15_v = XYL15.rearrange("p (c i g) -> p c i g", c=5, i=3)
                Mxy = M15[:, 0:6 * G]
                Mlog = M15[:, 6 * G:15 * G]          # masked logits, j-major
                Mlog_v = Mlog.rearrange("p (j i g) -> p j i g", j=3, i=3)
                E_v = E[:, :].rearrange("p (j i g) -> p j i g", j=3, i=3)
                S_v = S[:, :].rearrange("p (j g) -> p j g", j=3)

                # ---- DVE head: u = p + t ; masked xy+logits ----
                nc.vector.tensor_tensor(qs[:, :], P3, T3, ALU.add)
                nc.vector.tensor_tensor(M15_v, XYL15_v, t_b15, ALU.mult)

                # ---- ACT stream: prev chunk's ln(S), then this chunk ----
                if pending:
                    emit_lnS()
                nc.scalar.activation(qsq[:, :], qs[:, :], ACT.Square,
                                     bias=-1.0, scale=1.0)
                nc.scalar.activation(
                    jb[:, :], qsq[:, :], ACT.Ln, bias=1e-6, scale=1.0,
                    accum_out=cols[:, cb + COL_BCE:cb + COL_BCE + 1],
                )
                nc.scalar.activation(E[:, :], Mlog, ACT.Exp)

                # ---- wh: col_MW += sum t*ow (fused accum via STT) ----
                nc.vector.scalar_tensor_tensor(
                    jmw[:, :], W3, 1.0, T3, ALU.mult, ALU.mult,
                    accum_out=cols[:, cb + COL_MW:cb + COL_MW + 1],
                )
                # mt = ow * (t*tw)
                nc.vector.tensor_tensor(mt[:, :], W3, TWM3, ALU.mult)
                nc.scalar.activation(lm[:, :], mt[:, :], ACT.Ln, bias=1e-12)
                nc.scalar.activation(
                    jwh[:, :], lm[:, :], ACT.Exp, bias=LN2, scale=0.5,
                    accum_out=cols[:, cb + COL_TS2:cb + COL_TS2 + 1],
                )

                # ---- xy MSE ----
                nc.vector.tensor_tensor(exy[:, :], Mxy, TXY6, ALU.subtract)
                nc.scalar.activation(
                    jsq[:, :], exy[:, :], ACT.Square,
                    accum_out=cols[:, cb + COL_SQXY:cb + COL_SQXY + 1],
                )

                # ---- CE select + tail. For the last chunk, do the S-adds
                # first so the final ln(S) overlaps the sel ops instead of
                # trailing them.
                def emit_sel():
                    for i in range(3):
                        nc.vector.scalar_tensor_tensor(
                            jsel[:, :], TGT3, float(i), Mlog_v[:, :, i],
                            ALU.is_equal, ALU.mult,
                            accum_out=cols[:, cb + COL_SEL0 + i:
                                           cb + COL_SEL0 + i + 1],
                        )

                def emit_sadds():
                    nc.vector.tensor_tensor(
                        S_v, E_v[:, :, 0], E_v[:, :, 1], ALU.add)
                    nc.vector.tensor_tensor(S_v, S_v, E_v[:, :, 2], ALU.add)

                last = (c == len(chunks) - 1)
                if last:
                    emit_sadds()
                    pending.append((cb, S, jS))
                    emit_lnS()
                    emit_sel()
                else:
                    emit_sel()
                    emit_sadds()
                    pending.append((cb, S, jS))

            while pending:
                emit_lnS()

            nc.sync.dma_start(out=res_hbm[:, :], in_=cols[:, :])

    nc.compile()
    return nc


def _chunks_for(g_total: int):
    if g_total == 1024:
        return CHUNKS_FULL
    for n in (4, 2, 1):
        if g_total % n == 0:
            return (g_total // n,) * n
    return (g_total,)


def planarize(o_shard: np.ndarray, t_shard: np.ndarray, chunks) -> np.ndarray:
    """(nb,3,7)+(nb,3,5) f32 -> flat [P*NPL*g_total] bf16, chunk-blocked."""
    nbb = o_shard.shape[0]
    gt = nbb // P
    ob = o_shard.reshape(P, gt, 3, 7)
    tb = t_shard.reshape(P, gt, 3, 5)
    planes = np.empty((P, NPL, gt), dtype=NP_BF16)
    op = ob.transpose(0, 3, 2, 1)                     # (P, 7c, 3i, gt)
    planes[:, 0:9] = op[:, 0:3].reshape(P, 9, gt)     # p, x, y
    planes[:, 9:18] = (
        ob[:, :, :, 4:7].transpose(0, 3, 2, 1).reshape(P, 9, gt))  # l j-major
    planes[:, 18:21] = op[:, 3]                       # w
    tp = tb.transpose(0, 3, 2, 1)                     # (P, 5c, 3i, gt)
    planes[:, 21:30] = tp[:, 0:3].reshape(P, 9, gt)   # t, tx, ty
    planes[:, 30:33] = tp[:, 0] * tp[:, 3]            # t*tw
    planes[:, 33:36] = tb[:, :, :, 4].transpose(0, 2, 1)           # tgt_j
    parts = []
    g0 = 0
    for G in chunks:
        parts.append(np.ascontiguousarray(planes[:, :, g0:g0 + G]).reshape(P, -1))
        g0 += G
    return np.concatenate(parts, axis=1).ravel()


def make_in_maps(output: np.ndarray, target: np.ndarray, chunks):
    b = output.shape[0]
    nb = b // N_CORES
    in_maps = []
    for k in range(N_CORES):
        data = planarize(output[k * nb:(k + 1) * nb],
                         target[k * nb:(k + 1) * nb], chunks)
        in_maps.append({"data": data})
    return in_maps


def host_tw_sum(target: np.ndarray) -> float:
    """Pure-target partial sum folded in on the host: sum of tw (bf16-cast,
    matching what the device would have seen)."""
    return float(
        target[:, :, 3].astype(NP_BF16).astype(np.float64).sum())


def combine_results(res_list, n_chunks: int, b_total: int,
                    s_tw: float) -> np.float32:
    acc = np.zeros(NCOL, dtype=np.float64)
    for res in res_list:
        r = np.asarray(res).astype(np.float64).reshape(P, n_chunks, NCOL)
        acc += r.sum(axis=(0, 1))
    s_wh = acc[COL_MW] + s_tw - acc[COL_TS2]
    s_sel = acc[COL_SEL0] + acc[COL_SEL1] + acc[COL_SEL2]
    denom = 3.0 * b_total
    loss = 0.5 + (
        5.0 * acc[COL_SQXY] + 10.0 * s_wh - 0.25 * acc[COL_BCE]
        + 3.0 * (acc[COL_LSE] - s_sel)
    ) / denom
    return np.float32(loss)


_CACHED = {}


def _get_nc(nb: int):
    g_total = nb // P
    chunks = _chunks_for(g_total)
    key = (g_total, chunks)
    if key not in _CACHED:
        _CACHED[key] = (build_kernel(g_total, chunks), chunks)
    return _CACHED[key]


def run_on_cores(output: np.ndarray, target: np.ndarray, trace: bool = False):
    b = output.shape[0]
    nb = b // N_CORES
    nc, chunks = _get_nc(nb)
    in_maps = make_in_maps(output, target, chunks)
    results = run_bass_kernel_spmd(
        nc, in_maps, core_ids=list(range(N_CORES)), trace=trace
    )
    res_list = [r["res"] for r in results.results]
    return res_list, len(chunks), results


def kernel(output: np.ndarray, target: np.ndarray) -> np.ndarray:
    output = np.asarray(output, dtype=np.float32)
    target = np.asarray(target, dtype=np.float32)
    b = output.shape[0]
    res_list, n_chunks, _ = run_on_cores(output, target)
    return combine_results(res_list, n_chunks=n_chunks, b_total=b,
                           s_tw=host_tw_sum(target))
